# revision 1
# baseline (speedup 1.0000x reference)
"""Trainium2 Bass kernel for windowed (sparse) gated attention.

Problem (hardcoded): B=2, S=4096, D=128, DI=1024 (8 heads x 128), W=128.
For each query window i (of 32), keys/values come from windows i-1,i,i+1
(3W=384 keys, zero-padded at sequence edges), plus an additive [S,S] bias
read only on those diagonal bands; softmax; gated by sigmoid(x@Wg.T+bg);
output projection Wo.

Sharding: sequence-parallel. Core c owns query windows [4c, 4c+4) for both
batches / all heads; it receives a halo'd, pre-transposed slice of seq and
the 6 key-window bias bands it needs (with -1e30 on invalid positions), so
there is no inter-core communication. Output is returned transposed per
core ([B, D, 512]) and re-assembled on the host.

Device-side layout ("layout B"): attention scores are computed transposed,
simT[key, q] = biasT + kT.T @ qT, banded by key-window J in -1..4, with a
uniform 384-wide q band per J (invalid band columns carry bias=-1e30 so
exp() makes them exact zeros; they then contribute 0 to both the softmax
denominators and the attention-weighted values). Softmax denominators are
column sums obtained with a ones-vector matmul on the PE; normalization is
deferred to the gated output (divides commute with the linear AV / output
projection, and gating is elementwise), where the reciprocal is broadcast
across partitions on GPSIMD.
"""

import numpy as np

import concourse.bass as bass
import concourse.mybir as mybir
import concourse.tile as tile
from concourse import bacc

F32 = mybir.dt.float32
F32R = mybir.dt.float32r
F16 = mybir.dt.float16

B, S, D, DI, W, H, DH = 2, 4096, 128, 1024, 128, 8, 128
NCORES = 8
NWIN = S // W                 # 32 windows total
NW = NWIN // NCORES           # 4 query windows per core
SC = NW * W                   # 512 query positions per core
NJ = NW + 2                   # 6 key windows per core (with halo)
SL = NJ * W                   # 768 key positions per core
NEG = -1.0e30

# ---------------------------------------------------------------- config
CFG = dict(
    use_f32r=True,        # bitcast matmul operands to float32r (fast PE path)
    norm_on_pool=True,    # recip-broadcast + normalize on GPSIMD (else PE+DVE)
    fp16_attn=True,       # fp16 attn probs + v: exact diagonal sums/AV
    debug=False,          # dump b=0 intermediates to DRAM
    nrep=1,               # repeat whole compute inside the NEFF (benchmarking)
)


def _mdt():
    return F32R if CFG["use_f32r"] else F32


def _r(ap):
    return ap


# band start (in windows, within the core's 4 q-windows) for key window J
def _band_o(J):
    return min(max(J - 1, 0), 1)


# valid band-block range [kmin, kmax] for key window J (blocks with |J-w|<=1)
def _band_k(J):
    o = _band_o(J)
    ks = [k for k in range(3) if abs(J - (o + k)) <= 1 and 0 <= o + k < NW]
    return ks[0], ks[-1]


# ---------------------------------------------------------------- device
def _build_device(nc, t):
    """Emit the whole per-core program. t = dict of dram tensor handles."""
    AF = mybir.ActivationFunctionType
    ALU = mybir.AluOpType

    from contextlib import ExitStack

    with tile.TileContext(nc) as tc, ExitStack() as st:
        cpool = st.enter_context(tc.tile_pool(name="consts", bufs=1))
        wpool = st.enter_context(tc.tile_pool(name="weights", bufs=1))
        bpool = st.enter_context(tc.tile_pool(name="batch", bufs=1))
        apool = st.enter_context(tc.tile_pool(name="attn", bufs=2))
        opool = st.enter_context(tc.tile_pool(name="og", bufs=1))
        ypool = st.enter_context(tc.tile_pool(name="yout", bufs=2))
        ps3 = st.enter_context(tc.tile_pool(name="ps3", bufs=2, space="PSUM"))
        psA = st.enter_context(tc.tile_pool(name="psA", bufs=2, space="PSUM"))
        psB = st.enter_context(tc.tile_pool(name="psB", bufs=2, space="PSUM"))

        MDT = _mdt()
        # ---- first input slice, then weights in first-use order
        x0 = bpool.tile([128, SL], MDT, tag="x")
        nc.sync.dma_start(x0, t["xT"][0])
        wv = wpool.tile([128, DI], MDT, tag="wv")
        nc.scalar.dma_start(wv, t["wvT"][:])
        wq = wpool.tile([128, DI], MDT, tag="wq")
        nc.sync.dma_start(wq, t["wqT"][:])
        wg = wpool.tile([128, DI], MDT, tag="wg")
        nc.gpsimd.dma_start(wg, t["wgT"][:])
        wk = wpool.tile([128, DI], MDT, tag="wk")
        nc.sync.dma_start(wk, t["wkT"][:])
        bq8 = wpool.tile([128, 8], F32, tag="bq8")
        nc.gpsimd.dma_start(bq8, t["bq8"][:])
        bg8 = wpool.tile([128, 8], F32, tag="bg8")
        nc.scalar.dma_start(bg8, t["bg8"][:])
        ident = cpool.tile([128, 128], MDT, tag="ident")
        nc.gpsimd.dma_start(ident, t["ident"][:])
        ADT = F16 if CFG["fp16_attn"] else MDT
        ones_col = cpool.tile([128, 1], ADT, tag="ones_col")
        nc.gpsimd.dma_start(ones_col, t["ccol"][:])
        crow = cpool.tile([1, 640], MDT, tag="crow")
        nc.gpsimd.dma_start(crow, t["crow"][:])
        ones_row = crow[:, 0:128]
        zrow = crow[:, 128:640]
        z128 = crow[:, 128:256]
        wo = wpool.tile([128, 8, 128], MDT, tag="wo")
        nc.sync.dma_start(wo, t["woT"][:])

        rpool = st.enter_context(tc.tile_pool(name="rstage", bufs=3))

        for rep in range(CFG["nrep"]):
          og = [
            opool.tile([128, H, SC], MDT, tag=f"og{b}", name=f"og{b}_{rep}")
            for b in range(B)
          ]
          sums8s, recip8s = [], []
          for b in range(B):
              sums4 = [
                  opool.tile([2, 512], F32, tag=f"sums4_{b}{i}",
                             name=f"sums4_{b}{i}_{rep}")
                  for i in range(4)
              ]
              recip4 = [
                  opool.tile([2, 512], F32, tag=f"recip4_{b}{i}",
                             name=f"recip4_{b}{i}_{rep}")
                  for i in range(4)
              ]
              # ---- load inputs for this batch (x[0] prefetched above)
              if b == 0 and rep == 0:
                  x = x0
              else:
                  x = bpool.tile([128, SL], MDT, tag="x")
                  nc.sync.dma_start(x, t["xT"][b])
              bias = bpool.tile([128, NJ, 3 * W], MDT, tag="bias")
              nc.sync.dma_start(bias, t["biasT"][b])

              # ---- projections (all contract over D=128 on partitions)
              qT = bpool.tile([128, H, SC], MDT, tag="qT")
              gT = bpool.tile([128, H, SC], F32, tag="gT")
              kT = bpool.tile([128, H, SL], MDT, tag="kT")
              vv = bpool.tile([128, NJ, DI], ADT, tag="vv")
              xc = x[:, W : W + SC]  # center positions (the core's own queries)
              for sc_i in range(NJ):
                  xs = x[:, sc_i * 128 : (sc_i + 1) * 128]
                  pv = ps3.tile([128, 2, 512], F32, tag="ps3")
                  nc.tensor.matmul(pv[:, 0, :], _r(xs), _r(wv[:, 0:512]),
                                   start=True, stop=True)
                  nc.tensor.matmul(pv[:, 1, :], _r(xs), _r(wv[:, 512:1024]),
                                   start=True, stop=True)
                  pvv = pv.rearrange("p a b -> p (a b)")
                  nc.vector.tensor_copy(vv[:, sc_i, :], pvv[:, 0:DI])
              for c in range(8):
                  wq_c = wq[:, c * 128 : (c + 1) * 128]
                  pq = psA.tile([128, 512], F32, tag="psA")
                  nc.tensor.matmul(pq, _r(wq_c), _r(xc), start=True, stop=True)
                  # q = Wq@x + bq (bias per di-channel = per-partition here)
                  nc.vector.tensor_scalar_add(qT[:, c, :], pq, bq8[:, c : c + 1])

                  wg_c = wg[:, c * 128 : (c + 1) * 128]
                  pg = psB.tile([128, 512], F32, tag="psB")
                  nc.tensor.matmul(pg, _r(wg_c), _r(xc), start=True, stop=True)
                  # sigmoid(z) = 0.5*tanh(0.5 z)+0.5 : keep tanh here (same ACT
                  # table set as exp); +1 and *0.5 are folded into the gating
                  # multiply and Wo respectively.
                  nc.scalar.activation(
                      gT[:, c, :], pg, AF.Tanh, bias=bg8[:, c : c + 1], scale=0.5
                  )

                  wk_c = wk[:, c * 128 : (c + 1) * 128]
                  pk = ps3.tile([128, 2, 512], F32, tag="ps3")
                  nc.tensor.matmul(
                      pk[:, 0, :], _r(wk_c), _r(x[:, 0:512]), start=True, stop=True
                  )
                  nc.tensor.matmul(
                      pk[:, 1, 0:256], _r(wk_c), _r(x[:, 512:768]), start=True, stop=True
                  )
                  pkv = pk.rearrange("p a b -> p (a b)")
                  nc.scalar.copy(kT[:, c, :], pkv[:, 0:SL])


              if CFG["debug"] and b == 0:
                  for nm, tl in [("d_qT", qT), ("d_gT", gT), ("d_kT", kT),
                                 ("d_vv", vv)]:
                      nc.sync.dma_start(t[nm][:], tl[:])

              # ---- attention per head
              for h in range(H):
                  # attnT in diagonal slot layout [parity, slot//2] where
                  # slot = 3*w + jj covers (q-window w, key-window w+jj-1);
                  # each banded-exp write lands on a stride-2 slot run.
                  attnT = apool.tile([128, 2, 6, 128], ADT, tag="attnT")
                  for g2 in range(3):  # three groups of 2 key windows
                      psim = ps3.tile([128, 2, 512], F32, tag="ps3")
                      for j in range(2):
                          Jl = 2 * g2 + j          # storage index 0..5
                          J = Jl - 1               # key window -1..4
                          o = _band_o(J) * 128     # q-band start (elements)
                          out = psim[:, j, 0 : 3 * W]
                          # bias lands in PSUM via identity matmul, then the
                          # score matmul accumulates on top of it
                          nc.tensor.matmul(out, _r(ident), _r(bias[:, Jl, :]),
                                           start=True, stop=False)
                          kslice = kT[:, h, Jl * 128 : (Jl + 1) * 128]
                          qslice = qT[:, h, o : o + 3 * W]
                          nc.tensor.matmul(out, _r(kslice), _r(qslice),
                                           start=False, stop=True)
                      for j in range(2):
                          J = 2 * g2 + j - 1
                          o = _band_o(J)
                          kmin, kmax = _band_k(J)
                          nk = kmax - kmin + 1
                          w0 = o + kmin
                          s0 = 3 * w0 + (J - w0 + 1)
                          nc.scalar.activation(
                              attnT[:, s0 % 2, s0 // 2 : s0 // 2 + nk, :],
                              psim[:, j, kmin * 128 : (kmin + nk) * 128],
                              AF.Exp,
                          )

                  # softmax denominators + AV, exact diagonal accumulation
                  psums = psA.tile([128, 512], F32, tag="psA")
                  srow = psums[0:1, :]
                  poT = psB.tile([128, 512], F32, tag="psB")
                  for w in range(NW):
                      for jj in range(3):
                          sl = 3 * w + jj
                          a_sl = attnT[:, sl % 2, sl // 2, :]
                          nc.tensor.matmul(
                              srow[:, w * 128 : (w + 1) * 128],
                              ones_col, a_sl,
                              start=(jj == 0), stop=(jj == 2),
                          )
                          nc.tensor.matmul(
                              poT[:, w * 128 : (w + 1) * 128],
                              vv[:, w + jj, h * 128 : (h + 1) * 128], a_sl,
                              start=(jj == 0), stop=(jj == 2),
                          )
                  s1 = apool.tile([1, 512], F32, tag="s1")
                  if h % 2 == 0:
                      nc.scalar.copy(s1, srow)
                  else:
                      nc.vector.tensor_copy(s1, srow)
                  nc.sync.dma_start(sums4[h // 2][h % 2 : h % 2 + 1, :], s1)
                  if h % 2 == 1:
                      i = h // 2
                      rscr = rpool.tile([2, 512], F32, tag="rscr")
                      with nc.allow_low_precision(reason="softmax recip"):
                          nc.vector.reciprocal_approx_accurate(
                              recip4[i], sums4[i], rscr
                          )
                  # gate (unnormalized): og = (tanh_g + 1) * oT ; the 0.5 of
                  # the sigmoid identity lives in Wo (host-folded)
                  nc.vector.scalar_tensor_tensor(
                      og[b][:, h, :], gT[:, h, :], 1.0, poT, ALU.add, ALU.mult
                  )
                  if CFG["debug"] and b == 0 and h == 0:
                      nc.sync.dma_start(t["d_attnT"][:], attnT[:])
                      po_sb = apool.tile([128, 512], F32, tag="po_sb")
                      nc.vector.tensor_copy(po_sb, poT)
                      nc.sync.dma_start(t["d_oT"][:], po_sb)

              # ---- per-batch tail: normalize (recip done per half), project
              for h in range(H):
                  rb_p0 = rpool.tile([1, 512], MDT, tag="rb_p0")
                  nc.sync.dma_start(
                      rb_p0,
                      recip4[h // 2][h % 2 : h % 2 + 1, :].bitcast(MDT),
                  )
                  prb3 = ps3.tile([128, 2, 512], F32, tag="ps3")
                  prb = prb3[:, 0, :]
                  nc.tensor.matmul(prb, _r(ones_row), _r(rb_p0),
                                   start=True, stop=True)
                  nc.vector.tensor_tensor(
                      og[b][:, h, :], og[b][:, h, :], prb, ALU.mult
                  )

              pf = psA.tile([128, 512], F32, tag="psA")
              for c in range(8):
                  nc.tensor.matmul(
                      pf, _r(wo[:, c, :]), _r(og[b][:, c, :]),
                      start=(c == 0), stop=(c == 7),
                  )
              y = ypool.tile([128, 512], F32, tag="y")
              nc.scalar.copy(y, pf)
              nc.sync.dma_start(t["yT"][b], y)


# ---------------------------------------------------------------- build
_CACHE = {}


def _get_nc():
    key = tuple(sorted(CFG.items()))
    if _CACHE.get("key") == key:
        return _CACHE["nc"], _CACHE["t"]
    nc = bacc.Bacc(None, target_bir_lowering=False)
    t = dict(
        xT=nc.dram_tensor("xT", [B, 128, SL], _mdt(), kind="ExternalInput"),
        biasT=nc.dram_tensor("biasT", [B, 128, NJ, 3 * W], _mdt(),
                             kind="ExternalInput"),
        wqT=nc.dram_tensor("wqT", [128, DI], _mdt(), kind="ExternalInput"),
        wkT=nc.dram_tensor("wkT", [128, DI], _mdt(), kind="ExternalInput"),
        wvT=nc.dram_tensor("wvT", [128, DI], _mdt(), kind="ExternalInput"),
        wgT=nc.dram_tensor("wgT", [128, DI], _mdt(), kind="ExternalInput"),
        woT=nc.dram_tensor("woT", [128, 8, 128], _mdt(), kind="ExternalInput"),
        bq8=nc.dram_tensor("bq8", [128, 8], F32, kind="ExternalInput"),
        bg8=nc.dram_tensor("bg8", [128, 8], F32, kind="ExternalInput"),
        yT=nc.dram_tensor("yT", [B, 128, SC], F32, kind="ExternalOutput"),
        ident=nc.dram_tensor("ident", [128, 128], _mdt(), kind="ExternalInput"),
        ccol=nc.dram_tensor("ccol", [128, 1],
                            F16 if CFG["fp16_attn"] else _mdt(),
                            kind="ExternalInput"),
        crow=nc.dram_tensor("crow", [1, 640], _mdt(), kind="ExternalInput"),
    )
    if CFG["debug"]:
        for nm, shp in [("d_qT", [128, H, SC]), ("d_gT", [128, H, SC]),
                        ("d_kT", [128, H, SL]), ("d_vv", [128, NJ, DI]),
                        ("d_attnT", [128, NJ, 3 * W]), ("d_oT", [128, 512]),
                        ("d_sums16", [16, 512])]:
            t[nm] = nc.dram_tensor(nm, shp, F32, kind="ExternalOutput")
    _build_device(nc, t)
    nc.compile()
    _CACHE["nc"], _CACHE["t"], _CACHE["key"] = nc, t, key
    return nc, t


# ---------------------------------------------------------------- host
def _prep_shared(Wq, bq, Wkv, Wg, bg, Wo):
    scale = DH ** -0.5
    wqT = np.ascontiguousarray((Wq * scale).T, np.float32)          # [128,1024]
    wkT = np.ascontiguousarray(Wkv[:DI].T, np.float32)
    wvT = np.ascontiguousarray(Wkv[DI:].T, np.float32)
    wgT = np.ascontiguousarray(Wg.T, np.float32)
    # gating uses (tanh(0.5 z)+1) and final matmul absorbs the 0.5
    woT = np.ascontiguousarray(
        (0.5 * Wo).T.reshape(8, 128, 128).transpose(1, 0, 2), np.float32
    )                                                                # [128,8,128]
    bq8 = np.ascontiguousarray((bq * scale).reshape(8, 128).T, np.float32)
    bg8 = np.ascontiguousarray((bg * 0.5).reshape(8, 128).T, np.float32)
    ident = np.eye(128, dtype=np.float32)
    ccol = np.ones((128, 1),
                   np.float16 if CFG["fp16_attn"] else np.float32)
    crow = np.zeros((1, 640), np.float32)
    crow[0, :128] = 1.0
    return dict(wqT=wqT, wkT=wkT, wvT=wvT, wgT=wgT, woT=woT, bq8=bq8, bg8=bg8,
                ident=ident, ccol=ccol, crow=crow)


def _prep_core(c, seq, attn_bias):
    lo = c * SC - W
    hi = c * SC + SC + W
    xs = np.zeros((B, SL, D), np.float32)
    a, bnd = max(lo, 0), min(hi, S)
    xs[:, a - lo : bnd - lo, :] = seq[:, a:bnd, :]
    xT = np.ascontiguousarray(xs.transpose(0, 2, 1))                 # [B,128,768]

    br = attn_bias.reshape(B, NWIN, W, NWIN, W)
    biasT = np.full((B, NJ, W, 3 * W), NEG, np.float32)
    for Jl in range(NJ):
        J = Jl - 1
        gk = 4 * c + J                      # global key window
        if not (0 <= gk < NWIN):
            continue
        o = _band_o(J)
        for wb in range(3):                 # q windows o..o+2 (core-local)
            w = o + wb
            if abs(J - w) > 1:
                continue
            gq = 4 * c + w
            blk = br[:, gq, :, gk, :]       # [B, q(128), k(128)]
            biasT[:, Jl, :, wb * W : (wb + 1) * W] = blk.transpose(0, 2, 1)
    return xT, np.ascontiguousarray(biasT.transpose(0, 2, 1, 3))


def kernel(seq, mask, attn_bias, Wq, bq, Wkv, Wg, bg, Wo):
    from concourse.bass_utils import run_bass_kernel_spmd

    nc, _ = _get_nc()
    seq = np.asarray(seq, np.float32)
    attn_bias = np.asarray(attn_bias, np.float32)
    shared = _prep_shared(
        np.asarray(Wq, np.float32), np.asarray(bq, np.float32),
        np.asarray(Wkv, np.float32), np.asarray(Wg, np.float32),
        np.asarray(bg, np.float32), np.asarray(Wo, np.float32),
    )
    in_maps = []
    for c in range(NCORES):
        xT, biasT = _prep_core(c, seq, attn_bias)
        in_maps.append(dict(xT=xT, biasT=biasT, **shared))

    res = run_bass_kernel_spmd(nc, in_maps, core_ids=list(range(NCORES)))
    y = np.empty((B, S, D), np.float32)
    for c in range(NCORES):
        yT = res.results[c]["yT"]                                    # [B,128,512]
        y[:, c * SC : (c + 1) * SC, :] = yT.transpose(0, 2, 1)
    return y

